# revision 44
# baseline (speedup 1.0000x reference)
"""Multi-head attention TRN2 kernel, sharded over 8 NeuronCores.

Sharding: (batch, head-group) — core c handles batch c//4 and heads
(c%4)*4 .. (c%4)*4+3. Each core computes its 4 heads' attention plus its
partial output projection; the host sums the 4 partials per batch and adds bo.

v2 layout (vs the earlier transpose-on-device version):
  - q/k/v arrive HOST-pre-transposed as xT [8, 128, S] (d-major) so stage 0
    is pure projection matmuls (no PE transposes, no PSUM->SBUF copy pairs);
    q/k/v biases are folded in as K=1 ones-row matmuls.
  - K/Q projections are head-PAIR packed: kt2/qt2 [128, pair, S] hold head
    2p on partitions 0-63 and head 2p+1 on 64-127. The two heads' score
    matmuls then occupy disjoint PE row-groups (tile_position auto-derived
    from the base partitions) and can overlap in the array.
  - scores for a (pair, jt) land in one [128, 1024] PSUM tile (head A cols
    0-511 = bank 0, head B cols 512-1023 = bank 1); rel-pos bias (+mask,
    fp8) is folded on the PE via ident8 DoubleRow matmuls; ONE [128, 1024]
    exp per round keeps the ACT per-instruction overhead amortized.
  - softmax denominator: ones-column in V (cp row 64); cp is staged to SBUF
    right after the slab finishes (frees the PSUM bank), 1/denom via DVE
    reciprocal_approx_fast (ACT does nothing but Exp -> no table thrash),
    partition-broadcast via a DRAM bounce, applied by DVE into ctxT.
  - projections / out-projection / norm steps are drip-fed into the stage-1
    rounds through pending-work queues to keep the PE stream dense (HAM).
"""
import os
import sys

if "/opt/trn_rl_repo" not in sys.path:
    sys.path.insert(0, "/opt/trn_rl_repo")

DEBUG = os.environ.get("KDEBUG", "0") == "1"

from contextlib import ExitStack

import ml_dtypes
import numpy as np

B, S, D, H = 2, 2048, 1024, 16
HD = D // H          # 64
NCORES = 8
HPC = 4              # heads per core
NPAIR = 2            # head pairs per core
P = 128
ISLAB = 512
NJT = S // P         # 16
NISL = S // ISLAB    # 4
LAGR = 7             # ctx runs LAGR rounds behind scores
MASK_NEG = np.float32(-200.0)

_CACHE = {}


def _build():
    import concourse.bass as bass
    import concourse.mybir as mybir
    import concourse.tile as tile
    from concourse.tile import add_dep_helper
    from concourse import bacc

    f32 = mybir.dt.float32
    bf16 = mybir.dt.bfloat16
    f8 = mybir.dt.float8e4
    DR = mybir.MatmulPerfMode.DoubleRow
    EXP = mybir.ActivationFunctionType.Exp

    nc = bacc.Bacc(None, target_bir_lowering=False)

    xq = nc.declare_dram_parameter("xq", [8, P, S], bf16, isOutput=False)
    xk = nc.declare_dram_parameter("xk", [8, P, S], bf16, isOutput=False)
    xv = nc.declare_dram_parameter("xv", [8, P, S], bf16, isOutput=False)
    wq = nc.declare_dram_parameter("wq", [8, P, NPAIR, P], bf16, isOutput=False)
    wk = nc.declare_dram_parameter("wk", [8, P, NPAIR, P], bf16, isOutput=False)
    wv = nc.declare_dram_parameter("wv", [8, P, HPC * HD], bf16, isOutput=False)
    wo = nc.declare_dram_parameter("wo", [2, P, D], bf16, isOutput=False)
    bqk = nc.declare_dram_parameter("bqk", [1, 2, NPAIR, P], bf16,
                                    isOutput=False)
    bv_r = nc.declare_dram_parameter("bv_r", [1, HPC * HD], bf16, isOutput=False)
    # exp(rel-pos-bias + mask) per head: [h, isl, j%128, jt, ic]
    ebias = nc.declare_dram_parameter(
        "ebias", [HPC, NISL, P, NJT, ISLAB], bf16, isOutput=False)
    out_p = nc.declare_dram_parameter("out_p", [S, D], f32, isOutput=True)
    if DEBUG:
        dbg_stg = nc.declare_dram_parameter("dbg_stg", [HD + 1, ISLAB], f32,
                                            isOutput=True)
        dbg_rec = nc.declare_dram_parameter("dbg_rec", [1, ISLAB], f32,
                                            isOutput=True)
        dbg_bsb = nc.declare_dram_parameter("dbg_bsb", [HD, ISLAB], f32,
                                            isOutput=True)
        dbg_ctxT = nc.declare_dram_parameter("dbg_ctxT", [P, 2, S], bf16,
                                             isOutput=True)
        dbg_et = nc.declare_dram_parameter("dbg_et", [P, 2, ISLAB], bf16,
                                           isOutput=True)

    with tile.TileContext(nc) as tc, ExitStack() as big:
        consts = big.enter_context(tc.tile_pool(name="consts", bufs=1))
        persist = big.enter_context(tc.tile_pool(name="persist", bufs=1))

        ones_row = consts.tile([1, ISLAB], bf16)
        nc.vector.memset(ones_row, 1.0)
        bqk_sb = consts.tile([1, 2, NPAIR, P], bf16)
        nc.sync.dma_start(bqk_sb, bqk[:])
        bv_sb = consts.tile([1, HPC * HD], bf16)
        nc.sync.dma_start(bv_sb, bv_r[:])

        wq_sb = consts.tile([P, 8, NPAIR, P], bf16)
        wk_sb = consts.tile([P, 8, NPAIR, P], bf16)
        wv_sb = consts.tile([P, 8, HPC * HD], bf16)
        wo_sb = consts.tile([P, 2, D], bf16)

        qt2 = persist.tile([P, NPAIR, S], bf16)   # [d(pair-packed), pair, i]
        kt2 = persist.tile([P, NPAIR, S], bf16)   # [d(pair-packed), pair, j]
        v_full = persist.tile([P, NJT, HPC, HD + 1], bf16)
        ctxT = persist.tile([P, 2, S], bf16)      # [(h%2)*64+d, h//2, i]

        ones_col = consts.tile([P, 1], f32)
        nc.vector.memset(ones_col, 1.0)
        for jt in range(NJT):
            nc.vector.tensor_copy(
                v_full[:, jt, :, HD:HD + 1],
                ones_col[:, None, :].to_broadcast((P, HPC, 1)))

        # ---------------- pools -----------------------------------------
        xqp = big.enter_context(tc.tile_pool(name="xqp", bufs=8))
        xvp = big.enter_context(tc.tile_pool(name="xvp", bufs=8))
        sbias = big.enter_context(tc.tile_pool(name="sbias", bufs=6))
        sexp = big.enter_context(tc.tile_pool(name="sexp", bufs=9))
        sexr = big.enter_context(tc.tile_pool(name="sexr", bufs=1))
        sstg = big.enter_context(tc.tile_pool(name="sstg", bufs=2))
        snrm = big.enter_context(tc.tile_pool(name="snrm", bufs=2))
        dnrm = big.enter_context(tc.tile_pool(name="dnrm", bufs=2, space="DRAM"))
        so = big.enter_context(tc.tile_pool(name="so", bufs=2))

        sps = big.enter_context(tc.tile_pool(name="sps", bufs=2, space="PSUM"))
        cpa = big.enter_context(tc.tile_pool(name="cpa", bufs=1, space="PSUM"))
        cpb = big.enter_context(tc.tile_pool(name="cpb", bufs=1, space="PSUM"))
        ppp = big.enter_context(tc.tile_pool(name="ppp", bufs=1, space="PSUM"))
        opp = big.enter_context(tc.tile_pool(name="opp", bufs=1, space="PSUM"))

        nc.sync.dma_start(wq_sb, wq[:].rearrange("dk p r m -> p dk r m"))
        nc.sync.dma_start(wk_sb, wk[:].rearrange("dk p r m -> p dk r m"))
        nc.sync.dma_start(wv_sb, wv[:].rearrange("dk p m -> p dk m"))
        nc.sync.dma_start(wo_sb, wo[:].rearrange("kt p n -> p kt n"))

        def load_bias(h, isl, jtg):
            bt = sbias.tile([P, 4, ISLAB], bf16, tag="bt", name="bt")
            nc.sync.dma_start(bt, ebias[h, isl, :, jtg * 4:(jtg + 1) * 4])
            return bt

        bt_fifo = {h: [] for h in range(HPC)}
        for c in (0, 1):
            for h in (0, 1):
                bt_fifo[h].append(load_bias(h, 0, c))
        bt_cur = {}

        # ---------------- projection helpers ----------------------------
        xk_t, xq_t, xv_t = [None] * 8, [None] * 8, [None] * 8

        def load_x(which, dk, pool=None):
            pool, dram, arr = {
                "k": (pool, xk, xk_t), "q": (xqp, xq, xq_t),
                "v": (xvp, xv, xv_t)}[which]
            t = pool.tile([P, S], bf16, tag="x" + which)
            nc.sync.dma_start(t, dram[dk])
            arr[dk] = t

        def kq_proj(which, pair, sl):
            # one 512-wide slab of K or Q projection for one head pair
            w_sb = wk_sb if which == "k" else wq_sb
            x_t = xk_t if which == "k" else xq_t
            dst = kt2 if which == "k" else qt2
            brow = bqk_sb[0:1, 0 if which == "q" else 1, pair, :]
            pp = ppp.tile([P, ISLAB], f32, tag="pp", name="pp")
            for dk in range(8):
                nc.tensor.matmul(
                    pp, w_sb[:, dk, pair, :],
                    x_t[dk][:, sl * ISLAB:(sl + 1) * ISLAB],
                    start=(dk == 0), stop=False)
            nc.tensor.matmul(pp, brow, ones_row, start=False, stop=True)
            nc.vector.tensor_copy(
                dst[:, pair, sl * ISLAB:(sl + 1) * ISLAB], pp)

        def v_proj(jt):
            pp = ppp.tile([P, ISLAB], f32, tag="pp", name="pp")
            vp = pp[:, 0:HPC * HD]
            for dk in range(8):
                nc.tensor.matmul(
                    vp, xv_t[dk][:, jt * P:(jt + 1) * P], wv_sb[:, dk, :],
                    start=(dk == 0), stop=False)
            nc.tensor.matmul(vp, ones_row[:, 0:P], bv_sb, start=False, stop=True)
            nc.vector.tensor_copy(
                v_full[:, jt, :, :HD],
                vp.rearrange("p (h d) -> p h d", h=HPC))

        # ---------------- deferred-work machinery ------------------------
        pend = []   # FIFO of deferred emissions (projections, outproj)

        def drain(n):
            for _ in range(min(n, len(pend))):
                pend.pop(0)()

        def make_norm_steps(cp_t, h, isl):
            # stage cp (incl. denominator row) to SBUF first -> frees the
            # PSUM bank; then recip / broadcast-bounce / normalize.
            st = {}

            def s_stage():
                st["stg"] = sstg.tile([HD + 1, ISLAB], f32, tag="stg",
                                      name="stg")
                nc.vector.tensor_copy(st["stg"], cp_t)

            def s_dma1():
                # denominator row (partition 64) -> DRAM
                st["dn"] = dnrm.tile([1, ISLAB], f32, tag="dn", name="dn")
                nc.sync.dma_start(st["dn"], st["stg"][HD:HD + 1, :])

            def s_dma2():
                # broadcast-read the denominator into 64 partitions (base 0)
                dn = st["dn"]
                st["bsb"] = snrm.tile([HD, ISLAB], f32, tag="bsb", name="bsb")
                nc.sync.dma_start(st["bsb"], bass.AP(
                    tensor=dn.tensor, offset=dn.offset,
                    ap=[[0, HD]] + list(dn[0].ap)))

            def s_rec():
                # reciprocal_approx_fast mishandles base_partition != 0, so
                # run it after the broadcast where the tile starts at 0.
                st["rb"] = snrm.tile([HD, ISLAB], f32, tag="rb", name="rb")
                nc.vector.reciprocal_approx_fast(
                    out=st["rb"], in_=st["bsb"])

            def s_mul():
                nc.vector.tensor_mul(
                    ctxT[(h % 2) * 64:(h % 2) * 64 + 64, h // 2,
                         isl * ISLAB:(isl + 1) * ISLAB],
                    st["stg"][0:HD, :], st["rb"])
                if DEBUG and h == 0 and isl == 0:
                    nc.sync.dma_start(dbg_stg[:], st["stg"])
                    nc.sync.dma_start(dbg_rec[:], st["rb"][0:1, :])
                    nc.sync.dma_start(dbg_bsb[:], st["bsb"])

            return [s_stage, s_dma1, s_dma2, s_rec, s_mul]

        def make_outproj(isl):
            steps = []
            for it in range(isl * 4, isl * 4 + 4):
                for nh in range(2):
                    def run(it=it, nh=nh):
                        op = opp.tile([P, ISLAB], f32, tag="op", name="op")
                        for kt in range(2):
                            nc.tensor.matmul(
                                op, ctxT[:, kt, it * P:(it + 1) * P],
                                wo_sb[:, kt, nh * ISLAB:(nh + 1) * ISLAB],
                                start=(kt == 0), stop=(kt == 1))
                        ot = so.tile([P, ISLAB], f32, tag="ot", name="ot")
                        nc.vector.tensor_copy(ot, op)
                        nc.sync.dma_start(
                            out_p[it * P:(it + 1) * P,
                                  nh * ISLAB:(nh + 1) * ISLAB], ot)
                    steps.append(run)
            return steps

        # ---------------- prologue --------------------------------------
        with tc.tile_pool(name="xkp", bufs=8) as xkp:
            for dk in range(8):
                load_x("k", dk, pool=xkp)
            for dk in range(8):
                load_x("q", dk)
            for dk in range(8):
                load_x("v", dk)
            # K fully, Q islab 0, V jt 0..3; the rest drip-feeds via pend
            for sl in range(NISL):
                for pair in range(NPAIR):
                    kq_proj("k", pair, sl)
        for pair in range(NPAIR):
            kq_proj("q", pair, 0)
        for jt in range(4):
            v_proj(jt)
        for jt in range(4, NJT):
            pend.append(lambda jt=jt: v_proj(jt))

        # ---------------- stage 1: blocks of (islab, head-pair) ----------
        blocks = [(isl, pair) for isl in range(NISL) for pair in range(NPAIR)]
        carry = []            # ctx tail closures from previous block
        norm_pend = []        # norm steps from previous block
        for bi, (isl, pair) in enumerate(blocks):
            hA, hB = 2 * pair, 2 * pair + 1
            bt_cur = {hA: bt_fifo[hA].pop(0), hB: bt_fifo[hB].pop(0)}
            qt_A = qt2[0:64, pair, isl * ISLAB:(isl + 1) * ISLAB]
            qt_B = qt2[64:128, pair, isl * ISLAB:(isl + 1) * ISLAB]
            cpa_t = cpa.tile([HD + 1, ISLAB], f32, tag="cpa", name="cpa")
            cpb_t = cpb.tile([HD + 1, ISLAB], f32, tag="cpb", name="cpb")
            ets = [None] * NJT
            sc = [None] * NJT

            for jt in range(NJT):
                # 1) previous block's ctx tail (must precede its norm steps)
                if carry:
                    carry.pop(0)()
                # 2) previous block's norm pipeline, 2 steps per round
                if jt >= LAGR:
                    for _ in range(2):
                        if norm_pend:
                            norm_pend.pop(0)()
                # 3) drip-feed projections / out-projection
                drain(2)
                # 4) bias chunk rotation + prefetch (2 chunks ahead)
                if jt % 4 == 0 and jt > 0:
                    bt_cur = {hA: bt_fifo[hA].pop(0),
                              hB: bt_fifo[hB].pop(0)}
                if jt % 4 == 0:
                    cnext = jt // 4 + 2
                    if cnext < 4:
                        for h_ in (hA, hB):
                            bt_fifo[h_].append(load_bias(h_, isl, cnext))
                    elif bi + 1 < len(blocks):
                        isl_n, pair_n = blocks[bi + 1]
                        for h_ in (2 * pair_n, 2 * pair_n + 1):
                            bt_fifo[h_].append(
                                load_bias(h_, isl_n, cnext - 4))

                # 5) scores: one K=64 matmul per head of the pair
                sp = sps.tile([P, 2, ISLAB], f32, tag="sp", name="sp")
                smA = nc.tensor.matmul(
                    sp[:, 0, :], kt2[0:64, pair, jt * P:(jt + 1) * P],
                    qt_A, start=True, stop=True)
                nc.tensor.matmul(
                    sp[:, 1, :], kt2[64:128, pair, jt * P:(jt + 1) * P],
                    qt_B, start=True, stop=True)
                sc[jt] = smA
                # 6) one wide exp for both heads, then the multiplicative
                # rel-pos-bias+mask fold on DVE (all-SBUF bf16 -> 2x mode)
                etr = sexr.tile([P, 2, ISLAB], bf16, tag="etr", name="etr")
                nc.scalar.activation(etr, sp, EXP)
                et = sexp.tile([P, 2, ISLAB], bf16, tag="et", name="et")
                nc.vector.tensor_mul(et[:, 0, :], etr[:, 0, :],
                                     bt_cur[hA][:, jt % 4])
                nc.vector.tensor_mul(et[:, 1, :], etr[:, 1, :],
                                     bt_cur[hB][:, jt % 4])
                ets[jt] = et
                if DEBUG and bi == 0 and jt == 0:
                    nc.sync.dma_start(dbg_et[:], et)

                # 8) ctx, LAGR rounds behind
                if jt >= LAGR:
                    j2 = jt - LAGR
                    for h_, cp_t, half in ((hA, cpa_t, 0), (hB, cpb_t, 1)):
                        cmm = nc.tensor.matmul(
                            cp_t, v_full[:, j2, h_, :],
                            ets[j2][:, half, :],
                            start=(j2 == 0), stop=(j2 == NJT - 1))
                        add_dep_helper(sc[jt].ins, cmm.ins, sync=False,
                                       reason="preserve scores/ctx skew")

            # ctx tail -> start of next block; then norm steps
            carry = []
            for j2 in range(NJT - LAGR, NJT):
                def tail(j2=j2, ets=ets, cpa_t=cpa_t, cpb_t=cpb_t,
                         hA=hA, hB=hB):
                    for h_, cp_t, half in ((hA, cpa_t, 0), (hB, cpb_t, 1)):
                        nc.tensor.matmul(
                            cp_t, v_full[:, j2, h_, :],
                            ets[j2][:, half, :],
                            start=(j2 == 0), stop=(j2 == NJT - 1))
                carry.append(tail)
            nsA = make_norm_steps(cpa_t, hA, isl)
            nsB = make_norm_steps(cpb_t, hB, isl)
            norm_pend = [s for pairsteps in zip(nsA, nsB) for s in pairsteps]

            # q projection for the next islab during pair-1 blocks
            if pair == 1 and isl + 1 < NISL:
                for pr in range(NPAIR):
                    pend.append(lambda pr=pr, sl=isl + 1: kq_proj("q", pr, sl))
            # out-projection of islab isl-1 (norms finished a block ago)
            if pair == 1 and isl >= 1:
                pend.extend(make_outproj(isl - 1))

        # epilogue: leftover tails, norms, out-projections
        for run in carry:
            run()
        for step in norm_pend:
            step()
        drain(len(pend))
        for run in make_outproj(NISL - 1):
            run()
        if DEBUG:
            nc.sync.dma_start(dbg_ctxT[:], ctxT)

    nc.compile()
    return nc


def _get_nc():
    if "nc" not in _CACHE:
        _CACHE["nc"] = _build()
    return _CACHE["nc"]


def _prep_inputs(query, key, value, mask, relative_pos_bias,
                 Wq, bq, Wk, bk, Wv, bv, Wo, bo):
    f32 = np.float32
    bf = ml_dtypes.bfloat16
    f8 = ml_dtypes.float8_e4m3
    query = np.asarray(query, f32)
    key = np.asarray(key, f32)
    value = np.asarray(value, f32)
    rpb_T = np.ascontiguousarray(
        np.asarray(relative_pos_bias, f32).transpose(2, 0, 1))  # [H, j, i]
    mask_ji = np.asarray(mask)[:, 0].transpose(0, 2, 1)
    madd = np.where(mask_ji == 0, MASK_NEG, f32(0.0)).astype(f32)

    scale = f32(1.0 / np.sqrt(HD))
    Wq_s = np.asarray(Wq, f32) * scale
    bq_s = np.asarray(bq, f32) * scale
    Wk_f = np.asarray(Wk, f32)
    Wv_f = np.asarray(Wv, f32)
    Wo_f = np.asarray(Wo, f32)
    bk_f = np.asarray(bk, f32)
    bv_f = np.asarray(bv, f32)

    def xT(x):   # [S, D] -> [8, 128, S]
        return np.ascontiguousarray(x.T.reshape(8, P, S)).astype(bf)

    in_maps = []
    for c in range(NCORES):
        b = c // 4
        h0 = (c % 4) * HPC
        cols = slice(h0 * HD, (h0 + HPC) * HD)
        bias_hji = rpb_T[h0:h0 + HPC] + madd[b][None]   # [4, j, i]
        # exp(bias+mask): multiplicative fold; masked positions -> exactly 0
        # [pair, isl, jj, jt, hl, ic]; j = jt*128 + jj, i = isl*512 + ic
        eb = np.exp(bias_hji).reshape(HPC, NJT, P, NISL, ISLAB)
        eb = np.ascontiguousarray(eb.transpose(0, 3, 2, 1, 4)).astype(bf)
        in_maps.append({
            "xq": xT(query[b]),
            "xk": xT(key[b]),
            "xv": xT(value[b]),
            "wq": np.ascontiguousarray(
                Wq_s[:, cols].reshape(8, P, NPAIR, P)).astype(bf),
            "wk": np.ascontiguousarray(
                Wk_f[:, cols].reshape(8, P, NPAIR, P)).astype(bf),
            "wv": np.ascontiguousarray(
                Wv_f[:, cols].reshape(8, P, HPC * HD)).astype(bf),
            "wo": np.ascontiguousarray(
                Wo_f[cols, :].reshape(2, P, D)).astype(bf),
            "bqk": np.stack([bq_s[cols], bk_f[cols]]).reshape(
                1, 2, NPAIR, P).astype(bf),
            "bv_r": bv_f[cols].reshape(1, HPC * HD).astype(bf),
            "ebias": eb,
        })
    return in_maps


def run_sharded(run_kwargs=None, **inputs):
    """Build + run on 8 cores; returns (output, BassKernelResults)."""
    from concourse.bass_utils import run_bass_kernel_spmd

    nc = _get_nc()
    in_maps = _prep_inputs(**inputs)
    res = run_bass_kernel_spmd(nc, in_maps, list(range(NCORES)),
                               **(run_kwargs or {}))
    bo = np.asarray(inputs["bo"], np.float32)
    out = np.zeros((B, S, D), np.float32)
    for c in range(NCORES):
        out[c // 4] += res.results[c]["out_p"]
    out += bo[None, None, :]
    return out, res


def kernel(**inputs):
    out, _ = run_sharded(**inputs)
    return out


# revision 45
# speedup vs baseline: 1.1532x; 1.1532x over previous
"""Multi-head attention TRN2 kernel, sharded over 8 NeuronCores.

Sharding: (batch, head-group) — core c handles batch c//4 and heads
(c%4)*4 .. (c%4)*4+3. Each core computes its 4 heads' attention plus its
partial output projection; the host sums the 4 partials per batch and adds bo.

v2 layout (vs the earlier transpose-on-device version):
  - q/k/v arrive HOST-pre-transposed as xT [8, 128, S] (d-major) so stage 0
    is pure projection matmuls (no PE transposes, no PSUM->SBUF copy pairs);
    q/k/v biases are folded in as K=1 ones-row matmuls.
  - K/Q projections are head-PAIR packed: kt2/qt2 [128, pair, S] hold head
    2p on partitions 0-63 and head 2p+1 on 64-127. The two heads' score
    matmuls then occupy disjoint PE row-groups (tile_position auto-derived
    from the base partitions) and can overlap in the array.
  - scores for a (pair, jt) land in one [128, 1024] PSUM tile (head A cols
    0-511 = bank 0, head B cols 512-1023 = bank 1); rel-pos bias (+mask,
    fp8) is folded on the PE via ident8 DoubleRow matmuls; ONE [128, 1024]
    exp per round keeps the ACT per-instruction overhead amortized.
  - softmax denominator: ones-column in V (cp row 64); cp is staged to SBUF
    right after the slab finishes (frees the PSUM bank), 1/denom via DVE
    reciprocal_approx_fast (ACT does nothing but Exp -> no table thrash),
    partition-broadcast via a DRAM bounce, applied by DVE into ctxT.
  - projections / out-projection / norm steps are drip-fed into the stage-1
    rounds through pending-work queues to keep the PE stream dense (HAM).
"""
import os
import sys

if "/opt/trn_rl_repo" not in sys.path:
    sys.path.insert(0, "/opt/trn_rl_repo")

DEBUG = os.environ.get("KDEBUG", "0") == "1"

from contextlib import ExitStack

import ml_dtypes
import numpy as np

B, S, D, H = 2, 2048, 1024, 16
HD = D // H          # 64
NCORES = 8
HPC = 4              # heads per core
NPAIR = 2            # head pairs per core
P = 128
ISLAB = 512
NJT = S // P         # 16
NISL = S // ISLAB    # 4
LAGR = 7             # ctx runs LAGR rounds behind scores
MASK_NEG = np.float32(-200.0)

_CACHE = {}


def _build():
    import concourse.bass as bass
    import concourse.mybir as mybir
    import concourse.tile as tile
    from concourse.tile import add_dep_helper
    from concourse import bacc

    f32 = mybir.dt.float32
    bf16 = mybir.dt.bfloat16
    f8 = mybir.dt.float8e4
    DR = mybir.MatmulPerfMode.DoubleRow
    EXP = mybir.ActivationFunctionType.Exp

    nc = bacc.Bacc(None, target_bir_lowering=False)

    xq = nc.declare_dram_parameter("xq", [8, P, S], bf16, isOutput=False)
    xk = nc.declare_dram_parameter("xk", [8, P, S], bf16, isOutput=False)
    xv = nc.declare_dram_parameter("xv", [8, P, S], bf16, isOutput=False)
    wq = nc.declare_dram_parameter("wq", [8, P, NPAIR, P], bf16, isOutput=False)
    wk = nc.declare_dram_parameter("wk", [8, P, NPAIR, P], bf16, isOutput=False)
    wv = nc.declare_dram_parameter("wv", [8, P, HPC * HD], bf16, isOutput=False)
    wo = nc.declare_dram_parameter("wo", [2, P, D], bf16, isOutput=False)
    bqk = nc.declare_dram_parameter("bqk", [1, 2, NPAIR, P], bf16,
                                    isOutput=False)
    bv_r = nc.declare_dram_parameter("bv_r", [1, HPC * HD], bf16, isOutput=False)
    # exp(rel-pos-bias + mask) per head: [h, isl, j%128, jt, ic]
    ebias = nc.declare_dram_parameter(
        "ebias", [HPC, NISL, P, NJT, ISLAB], bf16, isOutput=False)
    out_p = nc.declare_dram_parameter("out_p", [S, D], f32, isOutput=True)
    if DEBUG:
        dbg_stg = nc.declare_dram_parameter("dbg_stg", [HD + 1, ISLAB], f32,
                                            isOutput=True)
        dbg_rec = nc.declare_dram_parameter("dbg_rec", [1, ISLAB], f32,
                                            isOutput=True)
        dbg_bsb = nc.declare_dram_parameter("dbg_bsb", [HD, ISLAB], f32,
                                            isOutput=True)
        dbg_ctxT = nc.declare_dram_parameter("dbg_ctxT", [P, 2, S], bf16,
                                             isOutput=True)
        dbg_et = nc.declare_dram_parameter("dbg_et", [P, 2, ISLAB], bf16,
                                           isOutput=True)

    with tile.TileContext(nc) as tc, ExitStack() as big:
        consts = big.enter_context(tc.tile_pool(name="consts", bufs=1))
        persist = big.enter_context(tc.tile_pool(name="persist", bufs=1))

        ones_row = consts.tile([1, ISLAB], bf16)
        nc.vector.memset(ones_row, 1.0)
        bqk_sb = consts.tile([1, 2, NPAIR, P], bf16)
        nc.sync.dma_start(bqk_sb, bqk[:])
        bv_sb = consts.tile([1, HPC * HD], bf16)
        nc.sync.dma_start(bv_sb, bv_r[:])

        wq_sb = consts.tile([P, 8, NPAIR, P], bf16)
        wk_sb = consts.tile([P, 8, NPAIR, P], bf16)
        wv_sb = consts.tile([P, 8, HPC * HD], bf16)
        wo_sb = consts.tile([P, 2, D], bf16)

        qt2 = persist.tile([P, NPAIR, S], bf16)   # [d(pair-packed), pair, i]
        kt2 = persist.tile([P, NPAIR, S], bf16)   # [d(pair-packed), pair, j]
        v_full = persist.tile([P, NJT, HPC, HD + 1], bf16)
        ctxT = persist.tile([P, 2, S], bf16)      # [(h%2)*64+d, h//2, i]

        ones_col = consts.tile([P, 1], f32)
        nc.vector.memset(ones_col, 1.0)
        for jt in range(NJT):
            nc.vector.tensor_copy(
                v_full[:, jt, :, HD:HD + 1],
                ones_col[:, None, :].to_broadcast((P, HPC, 1)))

        # ---------------- pools -----------------------------------------
        xqp = big.enter_context(tc.tile_pool(name="xqp", bufs=8))
        xvp = big.enter_context(tc.tile_pool(name="xvp", bufs=8))
        sbias = big.enter_context(tc.tile_pool(name="sbias", bufs=6))
        sexp = big.enter_context(tc.tile_pool(name="sexp", bufs=9))
        sexr = big.enter_context(tc.tile_pool(name="sexr", bufs=2))
        sstg = big.enter_context(tc.tile_pool(name="sstg", bufs=2))
        snrm = big.enter_context(tc.tile_pool(name="snrm", bufs=2))
        dnrm = big.enter_context(tc.tile_pool(name="dnrm", bufs=2, space="DRAM"))
        so = big.enter_context(tc.tile_pool(name="so", bufs=1))

        sps = big.enter_context(tc.tile_pool(name="sps", bufs=2, space="PSUM"))
        cpa = big.enter_context(tc.tile_pool(name="cpa", bufs=1, space="PSUM"))
        cpb = big.enter_context(tc.tile_pool(name="cpb", bufs=1, space="PSUM"))
        ppp = big.enter_context(tc.tile_pool(name="ppp", bufs=1, space="PSUM"))
        opp = big.enter_context(tc.tile_pool(name="opp", bufs=1, space="PSUM"))

        nc.sync.dma_start(wq_sb, wq[:].rearrange("dk p r m -> p dk r m"))
        nc.sync.dma_start(wk_sb, wk[:].rearrange("dk p r m -> p dk r m"))
        nc.sync.dma_start(wv_sb, wv[:].rearrange("dk p m -> p dk m"))
        nc.sync.dma_start(wo_sb, wo[:].rearrange("kt p n -> p kt n"))

        def load_bias(h, isl, jtg):
            bt = sbias.tile([P, 4, ISLAB], bf16, tag="bt", name="bt")
            nc.sync.dma_start(bt, ebias[h, isl, :, jtg * 4:(jtg + 1) * 4])
            return bt

        bt_fifo = {h: [] for h in range(HPC)}
        for c in (0, 1):
            for h in (0, 1):
                bt_fifo[h].append(load_bias(h, 0, c))
        bt_cur = {}

        # ---------------- projection helpers ----------------------------
        xk_t, xq_t, xv_t = [None] * 8, [None] * 8, [None] * 8

        def load_x(which, dk, pool=None):
            pool, dram, arr = {
                "k": (pool, xk, xk_t), "q": (xqp, xq, xq_t),
                "v": (xvp, xv, xv_t)}[which]
            t = pool.tile([P, S], bf16, tag="x" + which)
            nc.sync.dma_start(t, dram[dk])
            arr[dk] = t

        def kq_proj(which, pair, sl):
            # one 512-wide slab of K or Q projection for one head pair
            w_sb = wk_sb if which == "k" else wq_sb
            x_t = xk_t if which == "k" else xq_t
            dst = kt2 if which == "k" else qt2
            brow = bqk_sb[0:1, 0 if which == "q" else 1, pair, :]
            pp = ppp.tile([P, ISLAB], f32, tag="pp", name="pp")
            for dk in range(8):
                nc.tensor.matmul(
                    pp, w_sb[:, dk, pair, :],
                    x_t[dk][:, sl * ISLAB:(sl + 1) * ISLAB],
                    start=(dk == 0), stop=False)
            nc.tensor.matmul(pp, brow, ones_row, start=False, stop=True)
            nc.vector.tensor_copy(
                dst[:, pair, sl * ISLAB:(sl + 1) * ISLAB], pp)

        def v_proj(jt):
            pp = ppp.tile([P, ISLAB], f32, tag="pp", name="pp")
            vp = pp[:, 0:HPC * HD]
            for dk in range(8):
                nc.tensor.matmul(
                    vp, xv_t[dk][:, jt * P:(jt + 1) * P], wv_sb[:, dk, :],
                    start=(dk == 0), stop=False)
            nc.tensor.matmul(vp, ones_row[:, 0:P], bv_sb, start=False, stop=True)
            nc.vector.tensor_copy(
                v_full[:, jt, :, :HD],
                vp.rearrange("p (h d) -> p h d", h=HPC))

        # ---------------- deferred-work machinery ------------------------
        pend = []   # FIFO of deferred emissions (projections, outproj)

        def drain(n):
            for _ in range(min(n, len(pend))):
                pend.pop(0)()

        def make_norm_steps(cp_t, h, isl):
            # stage cp (incl. denominator row) to SBUF first -> frees the
            # PSUM bank; then recip / broadcast-bounce / normalize.
            st = {}

            def s_stage():
                st["stg"] = sstg.tile([HD + 1, ISLAB], f32, tag="stg",
                                      name="stg")
                nc.vector.tensor_copy(st["stg"], cp_t)

            def s_dma1():
                # denominator row (partition 64) -> DRAM
                st["dn"] = dnrm.tile([1, ISLAB], f32, tag="dn", name="dn")
                nc.sync.dma_start(st["dn"], st["stg"][HD:HD + 1, :])

            def s_dma2():
                # broadcast-read the denominator into 64 partitions (base 0)
                dn = st["dn"]
                st["bsb"] = snrm.tile([HD, ISLAB], f32, tag="bsb", name="bsb")
                nc.sync.dma_start(st["bsb"], bass.AP(
                    tensor=dn.tensor, offset=dn.offset,
                    ap=[[0, HD]] + list(dn[0].ap)))

            def s_rec():
                # reciprocal_approx_fast mishandles base_partition != 0, so
                # run it after the broadcast where the tile starts at 0.
                st["rb"] = snrm.tile([HD, ISLAB], f32, tag="rb", name="rb")
                nc.vector.reciprocal_approx_fast(
                    out=st["rb"], in_=st["bsb"])

            def s_mul():
                nc.vector.tensor_mul(
                    ctxT[(h % 2) * 64:(h % 2) * 64 + 64, h // 2,
                         isl * ISLAB:(isl + 1) * ISLAB],
                    st["stg"][0:HD, :], st["rb"])
                if DEBUG and h == 0 and isl == 0:
                    nc.sync.dma_start(dbg_stg[:], st["stg"])
                    nc.sync.dma_start(dbg_rec[:], st["rb"][0:1, :])
                    nc.sync.dma_start(dbg_bsb[:], st["bsb"])

            return [s_stage, s_dma1, s_dma2, s_rec, s_mul]

        def make_outproj(isl):
            steps = []
            for it in range(isl * 4, isl * 4 + 4):
                for nh in range(2):
                    def run(it=it, nh=nh):
                        op = opp.tile([P, ISLAB], f32, tag="op", name="op")
                        for kt in range(2):
                            nc.tensor.matmul(
                                op, ctxT[:, kt, it * P:(it + 1) * P],
                                wo_sb[:, kt, nh * ISLAB:(nh + 1) * ISLAB],
                                start=(kt == 0), stop=(kt == 1))
                        ot = so.tile([P, ISLAB], f32, tag="ot", name="ot")
                        nc.vector.tensor_copy(ot, op)
                        nc.sync.dma_start(
                            out_p[it * P:(it + 1) * P,
                                  nh * ISLAB:(nh + 1) * ISLAB], ot)
                    steps.append(run)
            return steps

        # ---------------- prologue --------------------------------------
        with tc.tile_pool(name="xkp", bufs=8) as xkp:
            for dk in range(8):
                load_x("k", dk, pool=xkp)
            for dk in range(8):
                load_x("q", dk)
            for dk in range(8):
                load_x("v", dk)
            # K fully, Q islab 0, V jt 0..3; the rest drip-feeds via pend
            for sl in range(NISL):
                for pair in range(NPAIR):
                    kq_proj("k", pair, sl)
        for pair in range(NPAIR):
            kq_proj("q", pair, 0)
        for jt in range(4):
            v_proj(jt)
        for jt in range(4, NJT):
            pend.append(lambda jt=jt: v_proj(jt))

        # ---------------- stage 1: blocks of (islab, head-pair) ----------
        blocks = [(isl, pair) for isl in range(NISL) for pair in range(NPAIR)]
        carry = []            # ctx tail closures from previous block
        norm_pend = []        # norm steps from previous block
        for bi, (isl, pair) in enumerate(blocks):
            hA, hB = 2 * pair, 2 * pair + 1
            bt_cur = {hA: bt_fifo[hA].pop(0), hB: bt_fifo[hB].pop(0)}
            qt_A = qt2[0:64, pair, isl * ISLAB:(isl + 1) * ISLAB]
            qt_B = qt2[64:128, pair, isl * ISLAB:(isl + 1) * ISLAB]
            cpa_t = cpa.tile([HD + 1, ISLAB], f32, tag="cpa", name="cpa")
            cpb_t = cpb.tile([HD + 1, ISLAB], f32, tag="cpb", name="cpb")
            ets = [None] * NJT
            sc = [None] * NJT

            for jt in range(NJT):
                # 1) previous block's ctx tail (must precede its norm steps)
                if carry:
                    carry.pop(0)()
                # 2) previous block's norm pipeline, 2 steps per round
                if jt >= LAGR:
                    for _ in range(2):
                        if norm_pend:
                            norm_pend.pop(0)()
                # 3) drip-feed projections / out-projection
                drain(2)
                # 4) bias chunk rotation + prefetch (2 chunks ahead)
                if jt % 4 == 0 and jt > 0:
                    bt_cur = {hA: bt_fifo[hA].pop(0),
                              hB: bt_fifo[hB].pop(0)}
                if jt % 4 == 0:
                    cnext = jt // 4 + 2
                    if cnext < 4:
                        for h_ in (hA, hB):
                            bt_fifo[h_].append(load_bias(h_, isl, cnext))
                    elif bi + 1 < len(blocks):
                        isl_n, pair_n = blocks[bi + 1]
                        for h_ in (2 * pair_n, 2 * pair_n + 1):
                            bt_fifo[h_].append(
                                load_bias(h_, isl_n, cnext - 4))

                # 5) scores: one K=64 matmul per head of the pair
                sp = sps.tile([P, 2, ISLAB], f32, tag="sp", name="sp")
                smA = nc.tensor.matmul(
                    sp[:, 0, :], kt2[0:64, pair, jt * P:(jt + 1) * P],
                    qt_A, start=True, stop=True)
                nc.tensor.matmul(
                    sp[:, 1, :], kt2[64:128, pair, jt * P:(jt + 1) * P],
                    qt_B, start=True, stop=True)
                sc[jt] = smA
                # 6) one wide exp for both heads, then the multiplicative
                # rel-pos-bias+mask fold on DVE (all-SBUF bf16 -> 2x mode)
                etr = sexr.tile([P, 2, ISLAB], bf16, tag="etr", name="etr")
                nc.scalar.activation(etr, sp, EXP)
                et = sexp.tile([P, 2, ISLAB], bf16, tag="et", name="et")
                nc.vector.tensor_mul(et[:, 0, :], etr[:, 0, :],
                                     bt_cur[hA][:, jt % 4])
                nc.vector.tensor_mul(et[:, 1, :], etr[:, 1, :],
                                     bt_cur[hB][:, jt % 4])
                ets[jt] = et
                if DEBUG and bi == 0 and jt == 0:
                    nc.sync.dma_start(dbg_et[:], et)

                # 8) ctx, LAGR rounds behind
                if jt >= LAGR:
                    j2 = jt - LAGR
                    for h_, cp_t, half in ((hA, cpa_t, 0), (hB, cpb_t, 1)):
                        cmm = nc.tensor.matmul(
                            cp_t, v_full[:, j2, h_, :],
                            ets[j2][:, half, :],
                            start=(j2 == 0), stop=(j2 == NJT - 1))
                        add_dep_helper(sc[jt].ins, cmm.ins, sync=False,
                                       reason="preserve scores/ctx skew")

            # ctx tail -> start of next block; then norm steps
            carry = []
            for j2 in range(NJT - LAGR, NJT):
                def tail(j2=j2, ets=ets, cpa_t=cpa_t, cpb_t=cpb_t,
                         hA=hA, hB=hB):
                    for h_, cp_t, half in ((hA, cpa_t, 0), (hB, cpb_t, 1)):
                        nc.tensor.matmul(
                            cp_t, v_full[:, j2, h_, :],
                            ets[j2][:, half, :],
                            start=(j2 == 0), stop=(j2 == NJT - 1))
                carry.append(tail)
            nsA = make_norm_steps(cpa_t, hA, isl)
            nsB = make_norm_steps(cpb_t, hB, isl)
            norm_pend = [s for pairsteps in zip(nsA, nsB) for s in pairsteps]

            # q projection for the next islab during pair-1 blocks
            if pair == 1 and isl + 1 < NISL:
                for pr in range(NPAIR):
                    pend.append(lambda pr=pr, sl=isl + 1: kq_proj("q", pr, sl))
            # out-projection of islab isl-1 (norms finished a block ago)
            if pair == 1 and isl >= 1:
                pend.extend(make_outproj(isl - 1))

        # epilogue: leftover tails, norms, out-projections
        for run in carry:
            run()
        for step in norm_pend:
            step()
        drain(len(pend))
        for run in make_outproj(NISL - 1):
            run()
        if DEBUG:
            nc.sync.dma_start(dbg_ctxT[:], ctxT)

    nc.compile()
    return nc


def _get_nc():
    if "nc" not in _CACHE:
        _CACHE["nc"] = _build()
    return _CACHE["nc"]


def _prep_inputs(query, key, value, mask, relative_pos_bias,
                 Wq, bq, Wk, bk, Wv, bv, Wo, bo):
    f32 = np.float32
    bf = ml_dtypes.bfloat16
    f8 = ml_dtypes.float8_e4m3
    query = np.asarray(query, f32)
    key = np.asarray(key, f32)
    value = np.asarray(value, f32)
    rpb_T = np.ascontiguousarray(
        np.asarray(relative_pos_bias, f32).transpose(2, 0, 1))  # [H, j, i]
    mask_ji = np.asarray(mask)[:, 0].transpose(0, 2, 1)
    madd = np.where(mask_ji == 0, MASK_NEG, f32(0.0)).astype(f32)

    scale = f32(1.0 / np.sqrt(HD))
    Wq_s = np.asarray(Wq, f32) * scale
    bq_s = np.asarray(bq, f32) * scale
    Wk_f = np.asarray(Wk, f32)
    Wv_f = np.asarray(Wv, f32)
    Wo_f = np.asarray(Wo, f32)
    bk_f = np.asarray(bk, f32)
    bv_f = np.asarray(bv, f32)

    def xT(x):   # [S, D] -> [8, 128, S]
        return np.ascontiguousarray(x.T.reshape(8, P, S)).astype(bf)

    in_maps = []
    for c in range(NCORES):
        b = c // 4
        h0 = (c % 4) * HPC
        cols = slice(h0 * HD, (h0 + HPC) * HD)
        bias_hji = rpb_T[h0:h0 + HPC] + madd[b][None]   # [4, j, i]
        # exp(bias+mask): multiplicative fold; masked positions -> exactly 0
        # [pair, isl, jj, jt, hl, ic]; j = jt*128 + jj, i = isl*512 + ic
        eb = np.exp(bias_hji).reshape(HPC, NJT, P, NISL, ISLAB)
        eb = np.ascontiguousarray(eb.transpose(0, 3, 2, 1, 4)).astype(bf)
        in_maps.append({
            "xq": xT(query[b]),
            "xk": xT(key[b]),
            "xv": xT(value[b]),
            "wq": np.ascontiguousarray(
                Wq_s[:, cols].reshape(8, P, NPAIR, P)).astype(bf),
            "wk": np.ascontiguousarray(
                Wk_f[:, cols].reshape(8, P, NPAIR, P)).astype(bf),
            "wv": np.ascontiguousarray(
                Wv_f[:, cols].reshape(8, P, HPC * HD)).astype(bf),
            "wo": np.ascontiguousarray(
                Wo_f[cols, :].reshape(2, P, D)).astype(bf),
            "bqk": np.stack([bq_s[cols], bk_f[cols]]).reshape(
                1, 2, NPAIR, P).astype(bf),
            "bv_r": bv_f[cols].reshape(1, HPC * HD).astype(bf),
            "ebias": eb,
        })
    return in_maps


def run_sharded(run_kwargs=None, **inputs):
    """Build + run on 8 cores; returns (output, BassKernelResults)."""
    from concourse.bass_utils import run_bass_kernel_spmd

    nc = _get_nc()
    in_maps = _prep_inputs(**inputs)
    res = run_bass_kernel_spmd(nc, in_maps, list(range(NCORES)),
                               **(run_kwargs or {}))
    bo = np.asarray(inputs["bo"], np.float32)
    out = np.zeros((B, S, D), np.float32)
    for c in range(NCORES):
        out[c // 4] += res.results[c]["out_p"]
    out += bo[None, None, :]
    return out, res


def kernel(**inputs):
    out, _ = run_sharded(**inputs)
    return out


# revision 46
# speedup vs baseline: 1.3030x; 1.1299x over previous
"""Multi-head attention TRN2 kernel, sharded over 8 NeuronCores.

Sharding: (batch, head-group) — core c handles batch c//4 and heads
(c%4)*4 .. (c%4)*4+3. Each core computes its 4 heads' attention plus its
partial output projection; the host sums the 4 partials per batch and adds bo.

v2 layout (vs the earlier transpose-on-device version):
  - q/k/v arrive HOST-pre-transposed as xT [8, 128, S] (d-major) so stage 0
    is pure projection matmuls (no PE transposes, no PSUM->SBUF copy pairs);
    q/k/v biases are folded in as K=1 ones-row matmuls.
  - K/Q projections are head-PAIR packed: kt2/qt2 [128, pair, S] hold head
    2p on partitions 0-63 and head 2p+1 on 64-127. The two heads' score
    matmuls then occupy disjoint PE row-groups (tile_position auto-derived
    from the base partitions) and can overlap in the array.
  - scores for a (pair, jt) land in one [128, 1024] PSUM tile (head A cols
    0-511 = bank 0, head B cols 512-1023 = bank 1); rel-pos bias (+mask,
    fp8) is folded on the PE via ident8 DoubleRow matmuls; ONE [128, 1024]
    exp per round keeps the ACT per-instruction overhead amortized.
  - softmax denominator: ones-column in V (cp row 64); cp is staged to SBUF
    right after the slab finishes (frees the PSUM bank), 1/denom via DVE
    reciprocal_approx_fast (ACT does nothing but Exp -> no table thrash),
    partition-broadcast via a DRAM bounce, applied by DVE into ctxT.
  - projections / out-projection / norm steps are drip-fed into the stage-1
    rounds through pending-work queues to keep the PE stream dense (HAM).
"""
import os
import sys

if "/opt/trn_rl_repo" not in sys.path:
    sys.path.insert(0, "/opt/trn_rl_repo")

DEBUG = os.environ.get("KDEBUG", "0") == "1"

from contextlib import ExitStack

import ml_dtypes
import numpy as np

B, S, D, H = 2, 2048, 1024, 16
HD = D // H          # 64
NCORES = 8
HPC = 4              # heads per core
NPAIR = 2            # head pairs per core
P = 128
ISLAB = 512
NJT = S // P         # 16
NISL = S // ISLAB    # 4
LAGR = 6             # ctx runs LAGR rounds behind scores
MASK_NEG = np.float32(-200.0)

_CACHE = {}


def _build():
    import concourse.bass as bass
    import concourse.mybir as mybir
    import concourse.tile as tile
    from concourse.tile import add_dep_helper
    from concourse import bacc

    f32 = mybir.dt.float32
    bf16 = mybir.dt.bfloat16
    f8 = mybir.dt.float8e4
    DR = mybir.MatmulPerfMode.DoubleRow
    EXP = mybir.ActivationFunctionType.Exp

    nc = bacc.Bacc(None, target_bir_lowering=False)

    xq = nc.declare_dram_parameter("xq", [8, P, S], bf16, isOutput=False)
    xk = nc.declare_dram_parameter("xk", [8, P, S], bf16, isOutput=False)
    xv = nc.declare_dram_parameter("xv", [8, P, S], bf16, isOutput=False)
    wq = nc.declare_dram_parameter("wq", [8, P, NPAIR, P], bf16, isOutput=False)
    wk = nc.declare_dram_parameter("wk", [8, P, NPAIR, P], bf16, isOutput=False)
    wv = nc.declare_dram_parameter("wv", [8, P, HPC * HD], bf16, isOutput=False)
    wo = nc.declare_dram_parameter("wo", [2, P, D], bf16, isOutput=False)
    bqk = nc.declare_dram_parameter("bqk", [1, 2, NPAIR, P], bf16,
                                    isOutput=False)
    bv_r = nc.declare_dram_parameter("bv_r", [1, HPC * HD], bf16, isOutput=False)
    # exp(rel-pos-bias + mask) per head: [h, isl, j%128, jt, ic]
    ebias = nc.declare_dram_parameter(
        "ebias", [HPC, NISL, P, NJT, ISLAB], bf16, isOutput=False)
    out_p = nc.declare_dram_parameter("out_p", [S, D], f32, isOutput=True)
    if DEBUG:
        dbg_stg = nc.declare_dram_parameter("dbg_stg", [HD + 1, ISLAB], f32,
                                            isOutput=True)
        dbg_rec = nc.declare_dram_parameter("dbg_rec", [1, ISLAB], f32,
                                            isOutput=True)
        dbg_bsb = nc.declare_dram_parameter("dbg_bsb", [HD, ISLAB], f32,
                                            isOutput=True)
        dbg_ctxT = nc.declare_dram_parameter("dbg_ctxT", [P, 2, S], bf16,
                                             isOutput=True)
        dbg_et = nc.declare_dram_parameter("dbg_et", [P, 2, ISLAB], bf16,
                                           isOutput=True)

    with tile.TileContext(nc) as tc, ExitStack() as big:
        consts = big.enter_context(tc.tile_pool(name="consts", bufs=1))
        persist = big.enter_context(tc.tile_pool(name="persist", bufs=1))

        ones_row = consts.tile([1, ISLAB], bf16)
        nc.vector.memset(ones_row, 1.0)
        bqk_sb = consts.tile([1, 2, NPAIR, P], bf16)
        nc.sync.dma_start(bqk_sb, bqk[:])
        bv_sb = consts.tile([1, HPC * HD], bf16)
        nc.sync.dma_start(bv_sb, bv_r[:])

        wq_sb = consts.tile([P, 8, NPAIR, P], bf16)
        wk_sb = consts.tile([P, 8, NPAIR, P], bf16)
        wv_sb = consts.tile([P, 8, HPC * HD], bf16)
        wo_sb = consts.tile([P, 2, D], bf16)

        qt2 = persist.tile([P, NPAIR, S], bf16)   # [d(pair-packed), pair, i]
        kt2 = persist.tile([P, NPAIR, S], bf16)   # [d(pair-packed), pair, j]
        v_full = persist.tile([P, NJT, HPC, HD + 1], bf16)
        ctxT = persist.tile([P, 2, S], bf16)      # [(h%2)*64+d, h//2, i]

        ones_col = consts.tile([P, 1], f32)
        nc.vector.memset(ones_col, 1.0)
        for jt in range(NJT):
            nc.vector.tensor_copy(
                v_full[:, jt, :, HD:HD + 1],
                ones_col[:, None, :].to_broadcast((P, HPC, 1)))

        # ---------------- pools -----------------------------------------
        xqp = big.enter_context(tc.tile_pool(name="xqp", bufs=8))
        xvp = big.enter_context(tc.tile_pool(name="xvp", bufs=8))
        sbias = big.enter_context(tc.tile_pool(name="sbias", bufs=6))
        sexp = big.enter_context(tc.tile_pool(name="sexp", bufs=8))
        sexr = big.enter_context(tc.tile_pool(name="sexr", bufs=2))
        sstg = big.enter_context(tc.tile_pool(name="sstg", bufs=2))
        snrm = big.enter_context(tc.tile_pool(name="snrm", bufs=2))
        dnrm = big.enter_context(tc.tile_pool(name="dnrm", bufs=2, space="DRAM"))
        so = big.enter_context(tc.tile_pool(name="so", bufs=2))

        sps = big.enter_context(tc.tile_pool(name="sps", bufs=2, space="PSUM"))
        cpa = big.enter_context(tc.tile_pool(name="cpa", bufs=1, space="PSUM"))
        cpb = big.enter_context(tc.tile_pool(name="cpb", bufs=1, space="PSUM"))
        ppp = big.enter_context(tc.tile_pool(name="ppp", bufs=1, space="PSUM"))
        opp = big.enter_context(tc.tile_pool(name="opp", bufs=1, space="PSUM"))

        nc.sync.dma_start(wq_sb, wq[:].rearrange("dk p r m -> p dk r m"))
        nc.sync.dma_start(wk_sb, wk[:].rearrange("dk p r m -> p dk r m"))
        nc.sync.dma_start(wv_sb, wv[:].rearrange("dk p m -> p dk m"))
        nc.sync.dma_start(wo_sb, wo[:].rearrange("kt p n -> p kt n"))

        def load_bias(h, isl, jtg):
            bt = sbias.tile([P, 4, ISLAB], bf16, tag="bt", name="bt")
            nc.sync.dma_start(bt, ebias[h, isl, :, jtg * 4:(jtg + 1) * 4])
            return bt

        bt_fifo = {h: [] for h in range(HPC)}
        for c in (0, 1):
            for h in (0, 1):
                bt_fifo[h].append(load_bias(h, 0, c))
        bt_cur = {}

        # ---------------- projection helpers ----------------------------
        xk_t, xq_t, xv_t = [None] * 8, [None] * 8, [None] * 8

        def load_x(which, dk, pool=None):
            pool, dram, arr = {
                "k": (pool, xk, xk_t), "q": (xqp, xq, xq_t),
                "v": (xvp, xv, xv_t)}[which]
            t = pool.tile([P, S], bf16, tag="x" + which)
            nc.sync.dma_start(t, dram[dk])
            arr[dk] = t

        def kq_proj(which, pair, sl):
            # one 512-wide slab of K or Q projection for one head pair
            w_sb = wk_sb if which == "k" else wq_sb
            x_t = xk_t if which == "k" else xq_t
            dst = kt2 if which == "k" else qt2
            brow = bqk_sb[0:1, 0 if which == "q" else 1, pair, :]
            pp = ppp.tile([P, ISLAB], f32, tag="pp", name="pp")
            for dk in range(8):
                nc.tensor.matmul(
                    pp, w_sb[:, dk, pair, :],
                    x_t[dk][:, sl * ISLAB:(sl + 1) * ISLAB],
                    start=(dk == 0), stop=False)
            nc.tensor.matmul(pp, brow, ones_row, start=False, stop=True)
            nc.vector.tensor_copy(
                dst[:, pair, sl * ISLAB:(sl + 1) * ISLAB], pp)

        def v_proj(jt):
            pp = ppp.tile([P, ISLAB], f32, tag="pp", name="pp")
            vp = pp[:, 0:HPC * HD]
            for dk in range(8):
                nc.tensor.matmul(
                    vp, xv_t[dk][:, jt * P:(jt + 1) * P], wv_sb[:, dk, :],
                    start=(dk == 0), stop=False)
            nc.tensor.matmul(vp, ones_row[:, 0:P], bv_sb, start=False, stop=True)
            nc.vector.tensor_copy(
                v_full[:, jt, :, :HD],
                vp.rearrange("p (h d) -> p h d", h=HPC))

        # ---------------- deferred-work machinery ------------------------
        pend = []   # FIFO of deferred emissions (projections, outproj)

        def drain(n):
            for _ in range(min(n, len(pend))):
                pend.pop(0)()

        def make_norm_steps(cp_t, h, isl):
            # stage cp (incl. denominator row) to SBUF first -> frees the
            # PSUM bank; then recip / broadcast-bounce / normalize.
            st = {}

            def s_stage():
                st["stg"] = sstg.tile([HD + 1, ISLAB], f32, tag="stg",
                                      name="stg")
                nc.vector.tensor_copy(st["stg"], cp_t)

            def s_dma1():
                # denominator row (partition 64) -> DRAM
                st["dn"] = dnrm.tile([1, ISLAB], f32, tag="dn", name="dn")
                nc.sync.dma_start(st["dn"], st["stg"][HD:HD + 1, :])

            def s_dma2():
                # broadcast-read the denominator into 64 partitions (base 0)
                dn = st["dn"]
                st["bsb"] = snrm.tile([HD, ISLAB], f32, tag="bsb", name="bsb")
                nc.sync.dma_start(st["bsb"], bass.AP(
                    tensor=dn.tensor, offset=dn.offset,
                    ap=[[0, HD]] + list(dn[0].ap)))

            def s_rec():
                # reciprocal_approx_fast mishandles base_partition != 0, so
                # run it after the broadcast where the tile starts at 0.
                st["rb"] = snrm.tile([HD, ISLAB], f32, tag="rb", name="rb")
                nc.vector.reciprocal_approx_fast(
                    out=st["rb"], in_=st["bsb"])

            def s_mul():
                nc.vector.tensor_mul(
                    ctxT[(h % 2) * 64:(h % 2) * 64 + 64, h // 2,
                         isl * ISLAB:(isl + 1) * ISLAB],
                    st["stg"][0:HD, :], st["rb"])
                if DEBUG and h == 0 and isl == 0:
                    nc.sync.dma_start(dbg_stg[:], st["stg"])
                    nc.sync.dma_start(dbg_rec[:], st["rb"][0:1, :])
                    nc.sync.dma_start(dbg_bsb[:], st["bsb"])

            return [s_stage, s_dma1, s_dma2, s_rec, s_mul]

        def make_outproj(isl):
            steps = []
            for it in range(isl * 4, isl * 4 + 4):
                for nh in range(2):
                    def run(it=it, nh=nh):
                        op = opp.tile([P, ISLAB], f32, tag="op", name="op")
                        for kt in range(2):
                            nc.tensor.matmul(
                                op, ctxT[:, kt, it * P:(it + 1) * P],
                                wo_sb[:, kt, nh * ISLAB:(nh + 1) * ISLAB],
                                start=(kt == 0), stop=(kt == 1))
                        ot = so.tile([P, ISLAB], f32, tag="ot", name="ot")
                        nc.vector.tensor_copy(ot, op)
                        nc.sync.dma_start(
                            out_p[it * P:(it + 1) * P,
                                  nh * ISLAB:(nh + 1) * ISLAB], ot)
                    steps.append(run)
            return steps

        # ---------------- prologue --------------------------------------
        with tc.tile_pool(name="xkp", bufs=8) as xkp:
            for dk in range(8):
                load_x("k", dk, pool=xkp)
            for dk in range(8):
                load_x("q", dk)
            for dk in range(8):
                load_x("v", dk)
            # K fully, Q islab 0, V jt 0..3; the rest drip-feeds via pend
            for sl in range(NISL):
                for pair in range(NPAIR):
                    kq_proj("k", pair, sl)
        for pair in range(NPAIR):
            kq_proj("q", pair, 0)
        for jt in range(4):
            v_proj(jt)
        for jt in range(4, NJT):
            pend.append(lambda jt=jt: v_proj(jt))

        # ---------------- stage 1: blocks of (islab, head-pair) ----------
        blocks = [(isl, pair) for isl in range(NISL) for pair in range(NPAIR)]
        carry = []            # ctx tail closures from previous block
        norm_pend = []        # norm steps from previous block
        for bi, (isl, pair) in enumerate(blocks):
            hA, hB = 2 * pair, 2 * pair + 1
            bt_cur = {hA: bt_fifo[hA].pop(0), hB: bt_fifo[hB].pop(0)}
            qt_A = qt2[0:64, pair, isl * ISLAB:(isl + 1) * ISLAB]
            qt_B = qt2[64:128, pair, isl * ISLAB:(isl + 1) * ISLAB]
            cpa_t = cpa.tile([HD + 1, ISLAB], f32, tag="cpa", name="cpa")
            cpb_t = cpb.tile([HD + 1, ISLAB], f32, tag="cpb", name="cpb")
            ets = [None] * NJT
            sc = [None] * NJT

            for jt in range(NJT):
                # 1) previous block's ctx tail (must precede its norm steps)
                if carry:
                    carry.pop(0)()
                # 2) previous block's norm pipeline, 2 steps per round
                if jt >= LAGR:
                    for _ in range(2):
                        if norm_pend:
                            norm_pend.pop(0)()
                # 3) drip-feed projections / out-projection
                drain(2)
                # 4) bias chunk rotation + prefetch (2 chunks ahead)
                if jt % 4 == 0 and jt > 0:
                    bt_cur = {hA: bt_fifo[hA].pop(0),
                              hB: bt_fifo[hB].pop(0)}
                if jt % 4 == 0:
                    cnext = jt // 4 + 2
                    if cnext < 4:
                        for h_ in (hA, hB):
                            bt_fifo[h_].append(load_bias(h_, isl, cnext))
                    elif bi + 1 < len(blocks):
                        isl_n, pair_n = blocks[bi + 1]
                        for h_ in (2 * pair_n, 2 * pair_n + 1):
                            bt_fifo[h_].append(
                                load_bias(h_, isl_n, cnext - 4))

                # 5) scores: one K=64 matmul per head of the pair
                sp = sps.tile([P, 2, ISLAB], f32, tag="sp", name="sp")
                smA = nc.tensor.matmul(
                    sp[:, 0, :], kt2[0:64, pair, jt * P:(jt + 1) * P],
                    qt_A, start=True, stop=True)
                nc.tensor.matmul(
                    sp[:, 1, :], kt2[64:128, pair, jt * P:(jt + 1) * P],
                    qt_B, start=True, stop=True)
                sc[jt] = smA
                # 6) one wide exp for both heads, then the multiplicative
                # rel-pos-bias+mask fold on DVE (all-SBUF bf16 -> 2x mode)
                etr = sexr.tile([P, 2, ISLAB], bf16, tag="etr", name="etr")
                nc.scalar.activation(etr, sp, EXP)
                et = sexp.tile([P, 2, ISLAB], bf16, tag="et", name="et")
                nc.vector.tensor_mul(et[:, 0, :], etr[:, 0, :],
                                     bt_cur[hA][:, jt % 4])
                nc.vector.tensor_mul(et[:, 1, :], etr[:, 1, :],
                                     bt_cur[hB][:, jt % 4])
                ets[jt] = et
                if DEBUG and bi == 0 and jt == 0:
                    nc.sync.dma_start(dbg_et[:], et)

                # 8) ctx, LAGR rounds behind
                if jt >= LAGR:
                    j2 = jt - LAGR
                    for h_, cp_t, half in ((hA, cpa_t, 0), (hB, cpb_t, 1)):
                        cmm = nc.tensor.matmul(
                            cp_t, v_full[:, j2, h_, :],
                            ets[j2][:, half, :],
                            start=(j2 == 0), stop=(j2 == NJT - 1))
                        add_dep_helper(sc[jt].ins, cmm.ins, sync=False,
                                       reason="preserve scores/ctx skew")

            # ctx tail -> start of next block; then norm steps
            carry = []
            for j2 in range(NJT - LAGR, NJT):
                def tail(j2=j2, ets=ets, cpa_t=cpa_t, cpb_t=cpb_t,
                         hA=hA, hB=hB):
                    for h_, cp_t, half in ((hA, cpa_t, 0), (hB, cpb_t, 1)):
                        nc.tensor.matmul(
                            cp_t, v_full[:, j2, h_, :],
                            ets[j2][:, half, :],
                            start=(j2 == 0), stop=(j2 == NJT - 1))
                carry.append(tail)
            nsA = make_norm_steps(cpa_t, hA, isl)
            nsB = make_norm_steps(cpb_t, hB, isl)
            norm_pend = [s for pairsteps in zip(nsA, nsB) for s in pairsteps]

            # q projection for the next islab during pair-1 blocks
            if pair == 1 and isl + 1 < NISL:
                for pr in range(NPAIR):
                    pend.append(lambda pr=pr, sl=isl + 1: kq_proj("q", pr, sl))
            # out-projection of islab isl-1 (norms finished a block ago)
            if pair == 1 and isl >= 1:
                pend.extend(make_outproj(isl - 1))

        # epilogue: leftover tails, norms, out-projections
        for run in carry:
            run()
        for step in norm_pend:
            step()
        drain(len(pend))
        for run in make_outproj(NISL - 1):
            run()
        if DEBUG:
            nc.sync.dma_start(dbg_ctxT[:], ctxT)

    nc.compile()
    return nc


def _get_nc():
    if "nc" not in _CACHE:
        _CACHE["nc"] = _build()
    return _CACHE["nc"]


def _prep_inputs(query, key, value, mask, relative_pos_bias,
                 Wq, bq, Wk, bk, Wv, bv, Wo, bo):
    f32 = np.float32
    bf = ml_dtypes.bfloat16
    f8 = ml_dtypes.float8_e4m3
    query = np.asarray(query, f32)
    key = np.asarray(key, f32)
    value = np.asarray(value, f32)
    rpb_T = np.ascontiguousarray(
        np.asarray(relative_pos_bias, f32).transpose(2, 0, 1))  # [H, j, i]
    mask_ji = np.asarray(mask)[:, 0].transpose(0, 2, 1)
    madd = np.where(mask_ji == 0, MASK_NEG, f32(0.0)).astype(f32)

    scale = f32(1.0 / np.sqrt(HD))
    Wq_s = np.asarray(Wq, f32) * scale
    bq_s = np.asarray(bq, f32) * scale
    Wk_f = np.asarray(Wk, f32)
    Wv_f = np.asarray(Wv, f32)
    Wo_f = np.asarray(Wo, f32)
    bk_f = np.asarray(bk, f32)
    bv_f = np.asarray(bv, f32)

    def xT(x):   # [S, D] -> [8, 128, S]
        return np.ascontiguousarray(x.T.reshape(8, P, S)).astype(bf)

    in_maps = []
    for c in range(NCORES):
        b = c // 4
        h0 = (c % 4) * HPC
        cols = slice(h0 * HD, (h0 + HPC) * HD)
        bias_hji = rpb_T[h0:h0 + HPC] + madd[b][None]   # [4, j, i]
        # exp(bias+mask): multiplicative fold; masked positions -> exactly 0
        # [pair, isl, jj, jt, hl, ic]; j = jt*128 + jj, i = isl*512 + ic
        eb = np.exp(bias_hji).reshape(HPC, NJT, P, NISL, ISLAB)
        eb = np.ascontiguousarray(eb.transpose(0, 3, 2, 1, 4)).astype(bf)
        in_maps.append({
            "xq": xT(query[b]),
            "xk": xT(key[b]),
            "xv": xT(value[b]),
            "wq": np.ascontiguousarray(
                Wq_s[:, cols].reshape(8, P, NPAIR, P)).astype(bf),
            "wk": np.ascontiguousarray(
                Wk_f[:, cols].reshape(8, P, NPAIR, P)).astype(bf),
            "wv": np.ascontiguousarray(
                Wv_f[:, cols].reshape(8, P, HPC * HD)).astype(bf),
            "wo": np.ascontiguousarray(
                Wo_f[cols, :].reshape(2, P, D)).astype(bf),
            "bqk": np.stack([bq_s[cols], bk_f[cols]]).reshape(
                1, 2, NPAIR, P).astype(bf),
            "bv_r": bv_f[cols].reshape(1, HPC * HD).astype(bf),
            "ebias": eb,
        })
    return in_maps


def run_sharded(run_kwargs=None, **inputs):
    """Build + run on 8 cores; returns (output, BassKernelResults)."""
    from concourse.bass_utils import run_bass_kernel_spmd

    nc = _get_nc()
    in_maps = _prep_inputs(**inputs)
    res = run_bass_kernel_spmd(nc, in_maps, list(range(NCORES)),
                               **(run_kwargs or {}))
    bo = np.asarray(inputs["bo"], np.float32)
    out = np.zeros((B, S, D), np.float32)
    for c in range(NCORES):
        out[c // 4] += res.results[c]["out_p"]
    out += bo[None, None, :]
    return out, res


def kernel(**inputs):
    out, _ = run_sharded(**inputs)
    return out
